# revision 16
# baseline (speedup 1.0000x reference)
"""DeepPoly ReLU transformer (back-substitution concretization) on 8 trn2 cores.

Math (exact rewrite of the reference):
    lb, ub = bounds;  plb, pub = last_bounds
    c = (plb+pub)/2, r = (pub-plb)/2
    s = W @ c,  q = |W| @ r       (identity holds for any sign of r)
    A = s - q  ( = max(W,0)@plb + min(W,0)@pub )
    B = s + q  ( = max(W,0)@pub + min(W,0)@plb )
    ind2 = lb>=0; ind3 = (ub>0)&(lb<0); ind4 = (ub>-lb)&ind3
    beta = 1 if ind2|ind4 else 0
    lmbda = 1 if ind2 else (ub/(ub-lb) if ind3 else 0)
    mu    = -lb*ub/(ub-lb) if ind3 else 0
    low = beta*(A + bias);  up = lmbda*(B + bias) + mu
    out_lb = max(beta*lb, low)
    out_ub = min(where(ind2|ind3, ub, 0), up)

Sharding: rows of W (output neurons) split across 8 cores, 1024 rows each.
Per core the host ships W transposed as fp16 [8192, 1024] (halves HBM
traffic vs fp32 and puts the contraction dim on partitions), so both
matvecs run on the TensorEngine: per [128, 1024] chunk, DVE computes
|Wt| (tensor_scalar abs_max, 4x mode), then PE accumulates
s = Wt.T @ c_chunk and q = |Wt|.T @ r_chunk into PSUM ([1,512] banks,
start/stop over the 64 chunks).  The [1, 1024] PSUM rows are scattered
to [128, 8] by a small SBUF->SBUF DMA and the exact fp32 mask epilogue
(same as the previous DVE version) finishes on [128, 8] tiles.
"""

import sys

sys.path.insert(0, "/opt/trn_rl_repo")

import numpy as np

N_CORES = 8
N = 8192
M = 8192


def _build(
    rows_per_core: int,
    m: int,
    concretize: bool,
    rep: int = 1,
    wbufs: int = 4,
    group: int = 8,
    qfp8: bool = True,
    ablate: str = "",
):
    import contextlib

    import concourse.tile as tile
    from concourse import bacc, mybir

    T = rows_per_core // 128
    assert rows_per_core % 128 == 0
    JC = m // 128  # number of 128-row contraction chunks

    # dual-fp8 ldweights requires the two k-tiles' weights to sit >=16B
    # apart in SBUF, so r columns are stored k-tile-major: [128, 2, RS]
    RS = max(JC // 2, 16)

    nc = bacc.Bacc("TRN2", target_bir_lowering=False, debug=False)
    f32 = mybir.dt.float32
    f16 = mybir.dt.float16
    f8 = mybir.dt.float8e4
    Alu = mybir.AluOpType
    Q8SCALE = 32.0  # shift |W| (sigma ~0.011) out of e4m3 subnormal range

    lbt = nc.dram_tensor("lbt", [128, T], f32, kind="ExternalInput").ap()
    ubt = nc.dram_tensor("ubt", [128, T], f32, kind="ExternalInput").ap()
    olb = nc.dram_tensor("olb", [128, T], f32, kind="ExternalOutput").ap()
    oub = nc.dram_tensor("oub", [128, T], f32, kind="ExternalOutput").ap()
    if concretize:
        # partition-major: wt[p, jc*rows + i] = W.T[jc*128 + p, i] so each
        # DMA group is 16KB-contiguous per partition (DMA efficiency)
        wt = nc.dram_tensor(
            "wt", [128, JC * rows_per_core], f16, kind="ExternalInput"
        ).ap()
        biast = nc.dram_tensor("biast", [128, T], f32, kind="ExternalInput").ap()
        ct = nc.dram_tensor("ct", [128, JC], f16, kind="ExternalInput").ap()
        rt = nc.dram_tensor("rt", [128, JC], f16, kind="ExternalInput").ap()
        rt8 = nc.dram_tensor("rt8", [128, 2 * RS], f8, kind="ExternalInput").ap()
        # DRAM scratch for the [1, rows] -> [128, T] partition scatter
        sdram = nc.dram_tensor("sdram", [T, 128], f32, kind="Internal").ap()
        qdram = nc.dram_tensor("qdram", [T, 128], f32, kind="Internal").ap()

    # psum banks per accumulator: free dim <= 512 fp32 (one 2KB bank) each
    bank_slices = [
        slice(b0, min(b0 + 512, rows_per_core))
        for b0 in range(0, rows_per_core, 512)
    ]
    NB = len(bank_slices)

    with tile.TileContext(nc) as tc:
        with (
            tc.tile_pool(name="wp", bufs=wbufs) as wp,
            tc.tile_pool(name="aq", bufs=wbufs) as aq,
            tc.tile_pool(name="ps", bufs=1, space="PSUM") as ps,
            tc.tile_pool(name="bc", bufs=1) as bc,
            tc.tile_pool(name="sm", bufs=1) as sm,
            tc.tile_pool(name="ep", bufs=24) as ep,
            tc.For_i(0, rep, 1) if rep > 1 else contextlib.nullcontext(),
        ):
            lb_s = sm.tile([128, T], f32, tag="lb")
            ub_s = sm.tile([128, T], f32, tag="ub")
            nc.sync.dma_start(lb_s[:], lbt[:])
            nc.sync.dma_start(ub_s[:], ubt[:])

            if concretize:
                ct_s = bc.tile([128, JC], f16, tag="ct")
                rt_s = bc.tile([128, JC], f16, tag="rt")
                nc.sync.dma_start(ct_s[:], ct[:])
                nc.sync.dma_start(rt_s[:], rt[:])
                rt8_s = bc.tile([128, 2 * RS], f8, tag="rt8")
                nc.sync.dma_start(rt8_s[:], rt8[:])
                rt8_v = rt8_s[:].rearrange("p (two s) -> p two s", two=2)
                bias_s = sm.tile([128, T], f32, tag="bias")
                nc.sync.dma_start(bias_s[:], biast[:])

                s_ps = [
                    ps.tile([1, cs.stop - cs.start], f32, name=f"sps{b}", tag=f"sps{b}")
                    for b, cs in enumerate(bank_slices)
                ]
                q_ps = [
                    ps.tile([1, cs.stop - cs.start], f32, name=f"qps{b}", tag=f"qps{b}")
                    for b, cs in enumerate(bank_slices)
                ]

                GF = group * rows_per_core  # free elems per DMA group
                for g in range(JC // group):
                    w = wp.tile([128, GF], f16, tag="w")
                    nc.sync.dma_start(w[:], wt[:, g * GF : (g + 1) * GF])
                    if ablate == "dma":
                        continue
                    if ablate != "smm":
                        a = aq.tile([128, GF], f16, tag="a")
                        # fp16 abs = clear sign bit (int16 view): DVE 4x mode
                        h = GF // 2
                        nc.vector.tensor_scalar(
                            a[:, 0:h].bitcast(mybir.dt.int16),
                            w[:, 0:h].bitcast(mybir.dt.int16),
                            0x7FFF, None, Alu.bitwise_and,
                        )
                        nc.vector.tensor_scalar(
                            a[:, h:GF].bitcast(mybir.dt.int16),
                            w[:, h:GF].bitcast(mybir.dt.int16),
                            0x7FFF, None, Alu.bitwise_and,
                        )
                        if ablate == "dmaabs":
                            continue
                    if qfp8 and ablate != "smm":
                        a8 = aq.tile([128, GF], f8, tag="a8")
                        nc.vector.tensor_scalar(
                            a8[:, 0:h], a[:, 0:h], Q8SCALE, None, Alu.mult
                        )
                        nc.vector.tensor_scalar(
                            a8[:, h:GF], a[:, h:GF], Q8SCALE, None, Alu.mult
                        )
                    for jcs in range(group):
                        jc = g * group + jcs
                        st, sp = jc == 0, jc == JC - 1
                        o = jcs * rows_per_core
                        for b, cs in enumerate(bank_slices):
                            nc.tensor.matmul(
                                s_ps[b][:], ct_s[:, jc : jc + 1],
                                w[:, o + cs.start : o + cs.stop],
                                start=st, stop=sp,
                            )
                        if not qfp8 or ablate == "smm":
                            qsrc = w if ablate == "smm" else a
                            for b, cs in enumerate(bank_slices):
                                nc.tensor.matmul(
                                    q_ps[b][:], rt_s[:, jc : jc + 1],
                                    qsrc[:, o + cs.start : o + cs.stop],
                                    start=st, stop=sp,
                                )
                    if qfp8 and ablate != "smm":
                        # DoubleRow: 2 contraction chunks per matmul, fp8 2x
                        for jp in range(group // 2):
                            jc = g * group + 2 * jp
                            jp2 = jc // 2
                            st, sp = jc == 0, jc == JC - 2
                            o = 2 * jp * rows_per_core
                            pair = a8[:, o : o + 2 * rows_per_core].rearrange(
                                "p (two i) -> p two i", two=2
                            )
                            lhs8 = rt8_v[:, :, jp2 : jp2 + 1]
                            for b, cs in enumerate(bank_slices):
                                nc.tensor.matmul(
                                    q_ps[b][:], lhs8,
                                    pair[:, :, cs.start : cs.stop],
                                    start=st, stop=sp,
                                    perf_mode=mybir.MatmulPerfMode.DoubleRow,
                                )

                if ablate in ("dma", "dmaabs"):
                    # accumulators never written; fake s/q for the epilogue
                    s_pt = sm.tile([128, T], f32, tag="spt")
                    q_pt = sm.tile([128, T], f32, tag="qpt")
                    nc.vector.memset(s_pt[:], 0.0)
                    nc.vector.memset(q_pt[:], 0.0)
                # PSUM [1, 512] rows -> SBUF [1, rows] -> scatter to [128, T]
                s_row = sm.tile([1, rows_per_core], f32, tag="srow")
                q_row = sm.tile([1, rows_per_core], f32, tag="qrow")
                if ablate in ("dma", "dmaabs"):
                    s_row = None  # sentinel: skip psum drain
                if s_row is not None:
                    for b, cs in enumerate(bank_slices):
                        nc.vector.tensor_copy(s_row[0:1, cs], s_ps[b][:])
                        nc.vector.tensor_copy(q_row[0:1, cs], q_ps[b][:])
                    s_pt = sm.tile([128, T], f32, tag="spt")
                    q_pt = sm.tile([128, T], f32, tag="qpt")
                    nc.sync.dma_start(sdram[:], s_row[0:1, :])
                    nc.sync.dma_start(qdram[:], q_row[0:1, :])
                    nc.sync.dma_start(s_pt[:], sdram.rearrange("t p -> p t"))
                    nc.sync.dma_start(q_pt[:], qdram.rearrange("t p -> p t"))

            # ---------------- epilogue (all fp32, [128, T]) ----------------
            def tt(a, b, op):
                o = ep.tile([128, T], f32)
                nc.vector.tensor_tensor(o[:], a[:], b[:], op=op)
                return o

            def ts(a, s1, op0, s2=None, op1=None):
                o = ep.tile([128, T], f32)
                if op1 is None:
                    nc.vector.tensor_scalar(o[:], a[:], s1, None, op0)
                else:
                    nc.vector.tensor_scalar(o[:], a[:], s1, s2, op0, op1)
                return o

            ind2 = ts(lb_s, 0.0, Alu.is_ge)
            ubpos = ts(ub_s, 0.0, Alu.is_gt)
            lbneg = ts(lb_s, 0.0, Alu.is_lt)
            ind3 = tt(ubpos, lbneg, Alu.mult)
            sumlu = tt(ub_s, lb_s, Alu.add)
            ind4p = ts(sumlu, 0.0, Alu.is_gt)
            ind4 = tt(ind4p, ind3, Alu.mult)
            beta = tt(ind2, ind4, Alu.max)

            lb_pre = tt(beta, lb_s, Alu.mult)
            i23 = tt(ind2, ind3, Alu.max)
            ub_pre = tt(ub_s, i23, Alu.mult)

            if concretize:
                diff = tt(ub_s, lb_s, Alu.subtract)
                dmask = tt(diff, ind3, Alu.mult)
                onemind3 = ts(ind3, -1.0, Alu.mult, 1.0, Alu.add)  # 1 - ind3
                diff_safe = tt(dmask, onemind3, Alu.add)
                rec = ep.tile([128, T], f32)
                nc.vector.reciprocal(rec[:], diff_safe[:])
                ubrec = tt(ub_s, rec, Alu.mult)
                lmb3 = tt(ubrec, ind3, Alu.mult)
                lmbda = tt(ind2, lmb3, Alu.add)
                negmu = tt(lmb3, lb_s, Alu.mult)  # = -mu

                if qfp8:
                    q_pt = ts(q_pt, 1.0 / Q8SCALE, Alu.mult)
                a_lo = tt(s_pt, q_pt, Alu.subtract)
                b_up = tt(s_pt, q_pt, Alu.add)
                a_b = tt(a_lo, bias_s, Alu.add)
                low = tt(a_b, beta, Alu.mult)
                b_b = tt(b_up, bias_s, Alu.add)
                b_l = tt(b_b, lmbda, Alu.mult)
                up = tt(b_l, negmu, Alu.subtract)

                out_lb = tt(lb_pre, low, Alu.max)
                out_ub = tt(ub_pre, up, Alu.min)
            else:
                out_lb = lb_pre
                out_ub = ub_pre

            nc.sync.dma_start(olb[:], out_lb[:])
            nc.sync.dma_start(oub[:], out_ub[:])

    nc.compile()
    return nc


_cache: dict = {}


def _get_nc(rows_per_core: int, m: int, concretize: bool, rep: int = 1, **kw):
    key = (rows_per_core, m, concretize, rep, tuple(sorted(kw.items())))
    if key not in _cache:
        _cache[key] = _build(rows_per_core, m, concretize, rep, **kw)
    return _cache[key]


def _make_in_maps(bounds, W, bias, last_bounds, concretize, n_cores):
    rows = W.shape[0] // n_cores if W is not None else bounds.shape[1] // n_cores
    T = rows // 128
    lb, ub = np.asarray(bounds[0], np.float32), np.asarray(bounds[1], np.float32)
    in_maps = []
    if concretize:
        m = W.shape[1]
        JC = m // 128
        plb = np.asarray(last_bounds[0], np.float64)
        pub = np.asarray(last_bounds[1], np.float64)
        c = ((plb + pub) * 0.5).astype(np.float32)
        r = ((pub - plb) * 0.5).astype(np.float32)
        # [128, JC] with element (p, jc) = v[jc*128 + p]
        import ml_dtypes
        ct = np.ascontiguousarray(c.astype(np.float16).reshape(JC, 128).T)
        rt = np.ascontiguousarray(r.astype(np.float16).reshape(JC, 128).T)
        r8col = r.astype(ml_dtypes.float8_e4m3).reshape(JC, 128).T  # [128, JC]
        RS = max(JC // 2, 16)
        rt8 = np.zeros((128, 2, RS), dtype=ml_dtypes.float8_e4m3)
        rt8[:, 0, : JC // 2] = r8col[:, 0::2]
        rt8[:, 1, : JC // 2] = r8col[:, 1::2]
        rt8 = np.ascontiguousarray(rt8.reshape(128, 2 * RS))
    for cix in range(n_cores):
        sl = slice(cix * rows, (cix + 1) * rows)
        im = {
            "lbt": np.ascontiguousarray(lb[sl].reshape(T, 128).T),
            "ubt": np.ascontiguousarray(ub[sl].reshape(T, 128).T),
        }
        if concretize:
            # [128, JC*rows]: wt[p, jc*rows + i] = W[sl][i, jc*128 + p]
            wt = W[sl].T.astype(np.float16)  # [m, rows]
            im["wt"] = np.ascontiguousarray(
                wt.reshape(JC, 128, rows).transpose(1, 0, 2).reshape(128, JC * rows)
            )
            im["biast"] = np.ascontiguousarray(
                np.asarray(bias, np.float32)[sl].reshape(T, 128).T
            )
            im["ct"] = ct
            im["rt"] = rt
            im["rt8"] = rt8
        in_maps.append(im)
    return in_maps


def _assemble(results, n_cores):
    outs = []
    for cix in range(n_cores):
        o_lb = results[cix]["olb"].T.reshape(-1)  # [T,128] -> rows t*128+p
        o_ub = results[cix]["oub"].T.reshape(-1)
        outs.append(np.stack([o_lb, o_ub]))
    return np.concatenate(outs, axis=1).astype(np.float32)


BEST = dict(wbufs=4, group=8)


def kernel(bounds, W, bias, last_bounds, back_sub_steps):
    from concourse.bass_utils import run_bass_kernel_spmd

    bounds = np.asarray(bounds)
    W = np.asarray(W)
    bias = np.asarray(bias)
    last_bounds = np.asarray(last_bounds)
    concretize = int(np.asarray(back_sub_steps)) > 0

    rows = W.shape[0] // N_CORES
    nc = _get_nc(rows, W.shape[1], concretize, **BEST)
    in_maps = _make_in_maps(
        bounds, W if concretize else None, bias, last_bounds, concretize, N_CORES
    )
    res = run_bass_kernel_spmd(nc, in_maps, list(range(N_CORES)))
    return _assemble(res.results, N_CORES)


if __name__ == "__main__":
    rng = np.random.default_rng(0)
    n, m = 1024, 2048  # small smoke (1 core slice = 128 rows)
    bounds = np.sort(rng.standard_normal((2, n)).astype(np.float32), axis=0)
    W = (rng.standard_normal((n, m)) / np.sqrt(m)).astype(np.float32)
    bias = rng.standard_normal(n).astype(np.float32)
    last_bounds = np.sort(rng.standard_normal((2, m)).astype(np.float32), axis=0)
    out = kernel(bounds, W, bias, last_bounds, 1)
    print(out.shape, out.dtype)


# revision 20
# speedup vs baseline: 1.1854x; 1.1854x over previous
"""DeepPoly ReLU transformer (back-substitution concretization) on 8 trn2 cores.

Math (exact rewrite of the reference):
    lb, ub = bounds;  plb, pub = last_bounds
    c = (plb+pub)/2, r = (pub-plb)/2
    s = W @ c,  q = |W| @ r       (identity holds for any sign of r)
    A = s - q  ( = max(W,0)@plb + min(W,0)@pub )
    B = s + q  ( = max(W,0)@pub + min(W,0)@plb )
    ind2 = lb>=0; ind3 = (ub>0)&(lb<0); ind4 = (ub>-lb)&ind3
    beta = 1 if ind2|ind4 else 0
    lmbda = 1 if ind2 else (ub/(ub-lb) if ind3 else 0)
    mu    = -lb*ub/(ub-lb) if ind3 else 0
    low = beta*(A + bias);  up = lmbda*(B + bias) + mu
    out_lb = max(beta*lb, low)
    out_ub = min(where(ind2|ind3, ub, 0), up)

Sharding: rows of W (output neurons) split across 8 cores, 1024 rows each.

Device pipeline per core (all engines in parallel, DMA-bound steady state):
  - host ships W.T as fp16 [128, JC*rows] partition-major so each 2MB DMA
    group lands 16KB-contiguous per partition (full HBM rate ~350GB/s/core)
  - s = W@c runs on the TensorEngine: per [128, rows] chunk,
    matmul(psum[1,512], lhsT=c_col[128,1], rhs=chunk) accumulated over chunks
  - q = |W|@r runs on the TensorEngine in fp8 DoubleRow (2 contraction
    rows/cycle): a8 = fp8e4(|32*W|) built half on DVE (int16-AND abs at 4x +
    mult-cast) and half on ScalarE (one Abs activation with scale);
    r columns live k-tile-major [128, 2, RS] because dual-fp8 ldweights
    needs the pair >=16B apart
  - PSUM accumulation is split in two halves (4 banks each) so the first
    half drains (PSUM->SBUF->DRAM bounce->[128,T] scatter) while the second
    half still accumulates; the serial tail is only the short combine
  - the lb/ub mask coefficients are computed during the first DMA group
"""

import sys

sys.path.insert(0, "/opt/trn_rl_repo")

import numpy as np

N_CORES = 8
N = 8192
M = 8192


def _build(
    rows_per_core: int,
    m: int,
    concretize: bool,
    rep: int = 1,
    wbufs: int = 5,
    group: int = 8,
    qfp8: bool = True,
    ablate: str = "",
):
    import contextlib

    import concourse.tile as tile
    from concourse import bacc, mybir

    T = rows_per_core // 128
    assert rows_per_core % 128 == 0
    JC = m // 128  # number of 128-row contraction chunks

    # dual-fp8 ldweights requires the two k-tiles' weights to sit >=16B
    # apart in SBUF, so r columns are stored k-tile-major: [128, 2, RS]
    RS = max(JC // 2, 16)

    nc = bacc.Bacc("TRN2", target_bir_lowering=False, debug=False)
    f32 = mybir.dt.float32
    f16 = mybir.dt.float16
    f8 = mybir.dt.float8e4
    Alu = mybir.AluOpType
    Act = mybir.ActivationFunctionType
    Q8SCALE = 32.0  # shift |W| (sigma ~0.011) out of e4m3 subnormal range

    lbt = nc.dram_tensor("lbt", [128, T], f32, kind="ExternalInput").ap()
    ubt = nc.dram_tensor("ubt", [128, T], f32, kind="ExternalInput").ap()
    ob = nc.dram_tensor("ob", [128, 2 * T], f32, kind="ExternalOutput").ap()
    if concretize:
        wt = nc.dram_tensor(
            "wt", [128, JC * rows_per_core], f16, kind="ExternalInput"
        ).ap()
        biast = nc.dram_tensor("biast", [128, T], f32, kind="ExternalInput").ap()
        ct = nc.dram_tensor("ct", [128, JC], f16, kind="ExternalInput").ap()
        rt = nc.dram_tensor("rt", [128, JC], f16, kind="ExternalInput").ap()
        rt8 = nc.dram_tensor("rt8", [128, 2 * RS], f8, kind="ExternalInput").ap()
        # DRAM scratch for the [2, rows] -> [128, 2T] partition scatter
        sq_dram = [
            nc.dram_tensor(f"sqd{i}", [2, T, 128], f32, kind="Internal").ap()
            for i in range(2)
        ]

    # psum banks per accumulator: free dim <= 512 fp32 (one 2KB bank) each
    bank_slices = [
        slice(b0, min(b0 + 512, rows_per_core))
        for b0 in range(0, rows_per_core, 512)
    ]
    # group boundaries; the final groups are smaller to shrink the
    # serial tail (last DMA -> abs -> matmul -> drain chain)
    gb = []
    pos = 0
    while pos < JC:
        rem = JC - pos
        if rem > group or rem <= max(group // 2, 2) or group <= 2:
            sz = min(group, rem)
        else:
            sz = rem - rem // 2
        gb.append((pos, pos + sz))
        pos += sz
    # two accumulation halves (psum sets), drained independently, split on a
    # group boundary nearest JC/2
    half_end = min((e for _, e in gb), key=lambda e: abs(e - JC // 2))
    if half_end == JC and len(gb) > 1:
        half_end = gb[-2][1]

    with tile.TileContext(nc) as tc:
        with (
            tc.tile_pool(name="wp", bufs=wbufs) as wp,
            tc.tile_pool(name="aq", bufs=wbufs) as aq,
            tc.tile_pool(name="ps", bufs=1, space="PSUM") as ps,
            tc.tile_pool(name="bc", bufs=1) as bc,
            tc.tile_pool(name="sm", bufs=1) as sm,
            tc.tile_pool(name="ep", bufs=24) as ep,
            tc.For_i(0, rep, 1) if rep > 1 else contextlib.nullcontext(),
        ):
            # kick off the W stream before anything else touches the queue
            if concretize:
                GF0 = (gb[0][1] - gb[0][0]) * rows_per_core
                w_first = wp.tile([128, GF0], f16, tag="w", name="w")
                nc.sync.dma_start(w_first[:], wt[:, 0:GF0])

            lb_s = sm.tile([128, T], f32, tag="lb")
            ub_s = sm.tile([128, T], f32, tag="ub")
            nc.scalar.dma_start(lb_s[:], lbt[:])
            nc.scalar.dma_start(ub_s[:], ubt[:])

            if concretize:
                ct_s = bc.tile([128, JC], f16, tag="ct")
                rt_s = bc.tile([128, JC], f16, tag="rt")
                nc.scalar.dma_start(ct_s[:], ct[:])
                nc.scalar.dma_start(rt_s[:], rt[:])
                rt8_s = bc.tile([128, 2 * RS], f8, tag="rt8")
                nc.scalar.dma_start(rt8_s[:], rt8[:])
                rt8_v = rt8_s[:].rearrange("p (two s) -> p two s", two=2)
                bias_s = sm.tile([128, T], f32, tag="bias")
                nc.scalar.dma_start(bias_s[:], biast[:])

            # ---- mask coefficients (lb/ub only) — run during group-0 DMA
            def tt(a, b, op):
                o = ep.tile([128, T], f32)
                nc.vector.tensor_tensor(o[:], a[:], b[:], op=op)
                return o

            def ts(a, s1, op0, s2=None, op1=None):
                o = ep.tile([128, T], f32)
                if op1 is None:
                    nc.vector.tensor_scalar(o[:], a[:], s1, None, op0)
                else:
                    nc.vector.tensor_scalar(o[:], a[:], s1, s2, op0, op1)
                return o

            ind2 = ts(lb_s, 0.0, Alu.is_ge)
            ubpos = ts(ub_s, 0.0, Alu.is_gt)
            lbneg = ts(lb_s, 0.0, Alu.is_lt)
            ind3 = tt(ubpos, lbneg, Alu.mult)
            sumlu = tt(ub_s, lb_s, Alu.add)
            ind4p = ts(sumlu, 0.0, Alu.is_gt)
            ind4 = tt(ind4p, ind3, Alu.mult)
            beta = tt(ind2, ind4, Alu.max)
            lb_pre = tt(beta, lb_s, Alu.mult)
            i23 = tt(ind2, ind3, Alu.max)
            ub_pre = tt(ub_s, i23, Alu.mult)
            if concretize:
                diff = tt(ub_s, lb_s, Alu.subtract)
                dmask = tt(diff, ind3, Alu.mult)
                onemind3 = ts(ind3, -1.0, Alu.mult, 1.0, Alu.add)  # 1 - ind3
                diff_safe = tt(dmask, onemind3, Alu.add)
                rec = ep.tile([128, T], f32)
                nc.vector.reciprocal(rec[:], diff_safe[:])
                ubrec = tt(ub_s, rec, Alu.mult)
                lmb3 = tt(ubrec, ind3, Alu.mult)
                lmbda = tt(ind2, lmb3, Alu.add)
                negmu = tt(lmb3, lb_s, Alu.mult)  # = -mu

            if concretize:
                # two psum half-sets: psum_sets[half] = (s_banks, q_banks)
                psum_sets = []
                for hx in range(2):
                    s_ps = [
                        ps.tile(
                            [1, cs.stop - cs.start], f32,
                            name=f"sps{hx}_{b}", tag=f"sps{hx}_{b}",
                        )
                        for b, cs in enumerate(bank_slices)
                    ]
                    q_ps = [
                        ps.tile(
                            [1, cs.stop - cs.start], f32,
                            name=f"qps{hx}_{b}", tag=f"qps{hx}_{b}",
                        )
                        for b, cs in enumerate(bank_slices)
                    ]
                    psum_sets.append((s_ps, q_ps))

                pt_tiles = {}  # (hx, 's'|'q') -> [128, T] tile

                def drain_half(hx):
                    s_ps, q_ps = psum_sets[hx]
                    # free halves = s then q; copies split DVE / ScalarE to
                    # shorten the serial drain (all on partition 0)
                    R = rows_per_core
                    sq_row = sm.tile(
                        [1, 2 * R], f32, name=f"sqr{hx}", tag=f"sqr{hx}"
                    )
                    for b, cs in enumerate(bank_slices):
                        nc.vector.tensor_copy(sq_row[0:1, cs], s_ps[b][:])
                        nc.scalar.activation(
                            sq_row[0:1, R + cs.start : R + cs.stop],
                            q_ps[b][:], Act.Copy,
                        )
                    sd = sq_dram[hx]
                    nc.scalar.dma_start(sd[:], sq_row[0:1, :])
                    sq_pt = sm.tile(
                        [128, 2, T], f32, name=f"sqpt{hx}", tag=f"sqpt{hx}"
                    )
                    nc.scalar.dma_start(
                        sq_pt[:], sd.rearrange("v t p -> p v t")
                    )
                    pt_tiles[(hx, "s")] = sq_pt[:, 0, :]
                    pt_tiles[(hx, "q")] = sq_pt[:, 1, :]

                for g, (j0, j1) in enumerate(gb):
                    gsz = j1 - j0
                    GF = gsz * rows_per_core
                    if g == 0:
                        w = w_first
                    else:
                        w = wp.tile([128, GF], f16, tag="w", name="w")
                        nc.sync.dma_start(
                            w[:],
                            wt[:, j0 * rows_per_core : j1 * rows_per_core],
                        )
                    if ablate == "dma":
                        continue
                    h = GF // 2
                    if qfp8:
                        # a8 = fp8e4(|32*w|): half on DVE, half on ScalarE
                        ah = aq.tile([128, h], f16, tag="ah", name="ah")
                        a8 = aq.tile([128, GF], f8, tag="a8", name="a8")
                        nc.vector.tensor_scalar(
                            ah[:].bitcast(mybir.dt.int16),
                            w[:, 0:h].bitcast(mybir.dt.int16),
                            0x7FFF, None, Alu.bitwise_and,
                        )
                        nc.vector.tensor_scalar(
                            a8[:, 0:h], ah[:], Q8SCALE, None, Alu.mult
                        )
                        nc.scalar.activation(
                            a8[:, h:GF], w[:, h:GF], Act.Abs, scale=Q8SCALE
                        )
                    else:
                        a = aq.tile([128, GF], f16, tag="a", name="a")
                        nc.vector.tensor_scalar(
                            a[:, 0:h].bitcast(mybir.dt.int16),
                            w[:, 0:h].bitcast(mybir.dt.int16),
                            0x7FFF, None, Alu.bitwise_and,
                        )
                        nc.vector.tensor_scalar(
                            a[:, h:GF].bitcast(mybir.dt.int16),
                            w[:, h:GF].bitcast(mybir.dt.int16),
                            0x7FFF, None, Alu.bitwise_and,
                        )
                    if ablate == "dmaabs":
                        continue

                    hx = 0 if j0 < half_end else 1
                    jlo = 0 if hx == 0 else half_end
                    jhi = half_end if hx == 0 else JC
                    s_ps, q_ps = psum_sets[hx]

                    for jcs in range(gsz):
                        jc = j0 + jcs
                        st, sp = jc == jlo, jc == jhi - 1
                        o = jcs * rows_per_core
                        for b, cs in enumerate(bank_slices):
                            nc.tensor.matmul(
                                s_ps[b][:], ct_s[:, jc : jc + 1],
                                w[:, o + cs.start : o + cs.stop],
                                start=st, stop=sp,
                            )
                        if not qfp8:
                            for b, cs in enumerate(bank_slices):
                                nc.tensor.matmul(
                                    q_ps[b][:], rt_s[:, jc : jc + 1],
                                    a[:, o + cs.start : o + cs.stop],
                                    start=st, stop=sp,
                                )
                    if qfp8:
                        for jp in range(gsz // 2):
                            jc = j0 + 2 * jp
                            jp2 = jc // 2
                            st = jc == jlo
                            sp = jc == jhi - 2
                            o = 2 * jp * rows_per_core
                            pair = a8[:, o : o + 2 * rows_per_core].rearrange(
                                "p (two i) -> p two i", two=2
                            )
                            lhs8 = rt8_v[:, :, jp2 : jp2 + 1]
                            for b, cs in enumerate(bank_slices):
                                nc.tensor.matmul(
                                    q_ps[b][:], lhs8,
                                    pair[:, :, cs.start : cs.stop],
                                    start=st, stop=sp,
                                    perf_mode=mybir.MatmulPerfMode.DoubleRow,
                                )
                    if ablate not in ("dma", "dmaabs"):
                        if j1 == half_end:
                            drain_half(0)
                        elif j1 == JC:
                            drain_half(1)

                if ablate in ("dma", "dmaabs"):
                    s_eff = sm.tile([128, T], f32, name="sfk", tag="sfk")
                    q_eff = sm.tile([128, T], f32, name="qfk", tag="qfk")
                    nc.vector.memset(s_eff[:], 0.0)
                    nc.vector.memset(q_eff[:], 0.0)
                else:
                    s_eff = tt(pt_tiles[(0, "s")], pt_tiles[(1, "s")], Alu.add)
                    q_eff = tt(pt_tiles[(0, "q")], pt_tiles[(1, "q")], Alu.add)
                if qfp8:
                    q_eff = ts(q_eff, 1.0 / Q8SCALE, Alu.mult)

                a_lo = tt(s_eff, q_eff, Alu.subtract)
                b_up = tt(s_eff, q_eff, Alu.add)
                a_b = tt(a_lo, bias_s, Alu.add)
                low = tt(a_b, beta, Alu.mult)
                b_b = tt(b_up, bias_s, Alu.add)
                b_l = tt(b_b, lmbda, Alu.mult)
                up = tt(b_l, negmu, Alu.subtract)

                ob_s = sm.tile([128, 2 * T], f32, tag="obs")
                nc.vector.tensor_tensor(
                    ob_s[:, 0:T], lb_pre[:], low[:], op=Alu.max
                )
                nc.vector.tensor_tensor(
                    ob_s[:, T : 2 * T], ub_pre[:], up[:], op=Alu.min
                )
            else:
                ob_s = sm.tile([128, 2 * T], f32, tag="obs")
                nc.vector.tensor_copy(ob_s[:, 0:T], lb_pre[:])
                nc.vector.tensor_copy(ob_s[:, T : 2 * T], ub_pre[:])

            nc.scalar.dma_start(ob[:], ob_s[:])

    nc.compile()
    return nc


_cache: dict = {}


def _get_nc(rows_per_core: int, m: int, concretize: bool, rep: int = 1, **kw):
    key = (rows_per_core, m, concretize, rep, tuple(sorted(kw.items())))
    if key not in _cache:
        _cache[key] = _build(rows_per_core, m, concretize, rep, **kw)
    return _cache[key]


def _make_in_maps(bounds, W, bias, last_bounds, concretize, n_cores):
    rows = W.shape[0] // n_cores if W is not None else bounds.shape[1] // n_cores
    T = rows // 128
    lb, ub = np.asarray(bounds[0], np.float32), np.asarray(bounds[1], np.float32)
    in_maps = []
    if concretize:
        import ml_dtypes

        m = W.shape[1]
        JC = m // 128
        plb = np.asarray(last_bounds[0], np.float64)
        pub = np.asarray(last_bounds[1], np.float64)
        c = ((plb + pub) * 0.5).astype(np.float32)
        r = ((pub - plb) * 0.5).astype(np.float32)
        # [128, JC] with element (p, jc) = v[jc*128 + p]
        ct = np.ascontiguousarray(c.astype(np.float16).reshape(JC, 128).T)
        rt = np.ascontiguousarray(r.astype(np.float16).reshape(JC, 128).T)
        r8col = r.astype(ml_dtypes.float8_e4m3).reshape(JC, 128).T  # [128, JC]
        RS = max(JC // 2, 16)
        rt8 = np.zeros((128, 2, RS), dtype=ml_dtypes.float8_e4m3)
        rt8[:, 0, : JC // 2] = r8col[:, 0::2]
        rt8[:, 1, : JC // 2] = r8col[:, 1::2]
        rt8 = np.ascontiguousarray(rt8.reshape(128, 2 * RS))
    for cix in range(n_cores):
        sl = slice(cix * rows, (cix + 1) * rows)
        im = {
            "lbt": np.ascontiguousarray(lb[sl].reshape(T, 128).T),
            "ubt": np.ascontiguousarray(ub[sl].reshape(T, 128).T),
        }
        if concretize:
            # [128, JC*rows]: wt[p, jc*rows + i] = W[sl][i, jc*128 + p]
            wtc = W[sl].T.astype(np.float16)  # [m, rows]
            im["wt"] = np.ascontiguousarray(
                wtc.reshape(JC, 128, rows).transpose(1, 0, 2).reshape(128, JC * rows)
            )
            im["biast"] = np.ascontiguousarray(
                np.asarray(bias, np.float32)[sl].reshape(T, 128).T
            )
            im["ct"] = ct
            im["rt"] = rt
            im["rt8"] = rt8
        in_maps.append(im)
    return in_maps


def _assemble(results, n_cores):
    outs = []
    for cix in range(n_cores):
        ob = results[cix]["ob"]  # [128, 2T]
        T2 = ob.shape[1] // 2
        o_lb = ob[:, 0:T2].T.reshape(-1)  # [T,128] -> rows t*128+p
        o_ub = ob[:, T2:].T.reshape(-1)
        outs.append(np.stack([o_lb, o_ub]))
    return np.concatenate(outs, axis=1).astype(np.float32)


BEST = dict(wbufs=5, group=8)


def kernel(bounds, W, bias, last_bounds, back_sub_steps):
    from concourse.bass_utils import run_bass_kernel_spmd

    bounds = np.asarray(bounds)
    W = np.asarray(W)
    bias = np.asarray(bias)
    last_bounds = np.asarray(last_bounds)
    concretize = int(np.asarray(back_sub_steps)) > 0

    rows = W.shape[0] // N_CORES
    nc = _get_nc(rows, W.shape[1], concretize, **BEST)
    in_maps = _make_in_maps(
        bounds, W if concretize else None, bias, last_bounds, concretize, N_CORES
    )
    res = run_bass_kernel_spmd(nc, in_maps, list(range(N_CORES)))
    return _assemble(res.results, N_CORES)


if __name__ == "__main__":
    rng = np.random.default_rng(0)
    n, m = 1024, 2048  # small smoke (1 core slice = 128 rows)
    bounds = np.sort(rng.standard_normal((2, n)).astype(np.float32), axis=0)
    W = (rng.standard_normal((n, m)) / np.sqrt(m)).astype(np.float32)
    bias = rng.standard_normal(n).astype(np.float32)
    last_bounds = np.sort(rng.standard_normal((2, m)).astype(np.float32), axis=0)
    out = kernel(bounds, W, bias, last_bounds, 1)
    print(out.shape, out.dtype)
